# revision 14
# baseline (speedup 1.0000x reference)
"""Trainium2 Bass kernel for nn_DynamicAggRecModel (gather + per-item MLP +
weighted pooling + rating MLP), data-parallel over batch on 8 NeuronCores.

Device layout (per core, Bc = 2048 batch rows):
  G = Bc/16 groups of 16 batch rows; T = 7 history tiles per group (H = 50
  padded to 56 = 7*8). History tile (g, t) places item (b = 16g + p%16,
  h = 8t + p//16) on partition p. Per tile:
    y = featT_tile.T @ Wf_bot            (PE, items on partitions)
    y += table2[idx]                     (indirect DMA gather, CCE add)
    scaled = max(y, 0) * w               (DVE, per-partition scalar)
    user_psum[16, 64] += comb.T @ scaled (PE, contracts h within each b)
  where table2 = embed_table @ fusion_w[:64] + fusion_b (folded on host) and
  w = rating - 3 (padded h has w = 0). user_vec = user_psum * 1/sum|w|, then
  PE-transposed into x^T[128, batch] columns together with the target rep,
  and the 3-layer MLP runs with batch on the moving dim.

PERF NOTES (cost-model ~970us/core; Pool engine 947us of that):
  The critical path is SWDGE descriptor generation for the 912 indirect
  gathers (~994ns fixed per instruction). Measured dead ends on this
  toolchain, for future reference:
  - Multi-column gathers (idx [128, N]) are broken in walrus: a flat dest
    consumes ONE index per contiguous dest run and reads N *consecutive*
    table rows; a padded-stride 3D dest leaves ~99% of runs unwritten
    (probe: 889/896 holes); a plain 3D dest AP hard-crashes NRT. A batched
    variant (one gather per 56 cols, models at 166us/core) is sim-correct
    but HW-wrong; revisit if the toolchain's indirect unroll is fixed.
  - Scheduling/buffer knobs are flat (makespan pinned at Pool busy).
  - Remaining legal headroom: merge the t=6 padding tails across 4 groups
    (912 -> ~816 gathers, ~-10%); route gathers across qPoolDynamic{0..3}
    with Bass(num_swdge_queues=4) *if* Tile's same-queue FIFO wait elision
    is audited; SBUF-resident table (needs the 192KB/partition cap raised).
"""

import numpy as np
import ml_dtypes

import concourse.bass as bass
import concourse.tile as tile
import concourse.mybir as mybir
from concourse.bass import IndirectOffsetOnAxis
from concourse.vector_clock import ScopedClock
from concourse.bass_utils import run_bass_kernel_spmd

F32 = mybir.dt.float32
BF16 = mybir.dt.bfloat16
I32 = mybir.dt.int32
AF = mybir.ActivationFunctionType
ALU = mybir.AluOpType
bf16 = ml_dtypes.bfloat16

N_CORES = 8
B = 16384
H = 50
V = 100000
Bc = B // N_CORES
G = Bc // 16
T = 7
K = Bc // 128
GPK = G // K

# ---------------------------------------------------------------------------
# Workarounds: this walrus build supports at most ONE sync-wait command per
# instruction. Split Tile's aggregated tail-drain waits (and any other
# instruction that accumulated >1 waits) into per-wait nops on the same
# engine.

_MAX_WAITS = 1


def _drain_and_barrier_split(self, tick_clock, wait_clock):
    nop = self.nc.sync.nop()
    wait_clock.add_sem_waits(nop.ins, ScopedClock({None: tick_clock.global_clock}))
    si = nop.ins.sync_info
    waits = list(si.on_wait) if si is not None else []
    if len(waits) > _MAX_WAITS:
        nop.ins.sync_info = mybir.SyncInfo(
            on_wait=waits[:_MAX_WAITS], on_update=list(si.on_update))
        for k in range(_MAX_WAITS, len(waits), _MAX_WAITS):
            extra = self.nc.sync.nop()
            extra.ins.sync_info = mybir.SyncInfo(
                on_wait=waits[k:k + _MAX_WAITS], on_update=[])
    self.nc.sync.drain()
    self.nc.all_engine_barrier()
    assert self.sems is not None
    popped = self.nc._tile_sem_poison_stack.pop()
    assert popped is self._sem_poison
    self.nc.clear_and_free_semaphores(list(self.sems.allocated().values()))
    self.nc.all_engine_barrier()


tile.TileContext._drain_and_barrier = _drain_and_barrier_split


def _split_excess_waits(nc):
    n = 0
    for f in nc.m.functions:
        for blk in f.blocks:
            insts = blk.instructions
            out = []
            changed = False
            for inst in insts:
                si = inst.sync_info
                waits = list(si.on_wait) if si is not None else []
                if len(waits) > _MAX_WAITS:
                    changed = True
                    for k in range(0, len(waits) - _MAX_WAITS, _MAX_WAITS):
                        nop = mybir.InstNoOp(
                            name=f"WSPL-{n}", engine=inst.engine,
                            sync_info=mybir.SyncInfo(
                                on_wait=waits[k:k + _MAX_WAITS], on_update=[]),
                        )
                        n += 1
                        out.append(nop)
                    inst.sync_info = mybir.SyncInfo(
                        on_wait=waits[len(waits) - _MAX_WAITS:],
                        on_update=list(si.on_update))
                out.append(inst)
            if changed:
                blk.instructions = out
    return n


# ---------------------------------------------------------------------------
# Device program


BG = 4  # groups per featsT DMA

_RR = [0]


def _rr_queue():
    q = _RR[0] % 4
    _RR[0] += 1
    return f"qPoolDynamic{q or ''}"
GS = 4  # groups per history gather


def build_kernel(nc, io, Bc=Bc, do_gather=True):
    G = Bc // 16
    K = Bc // 128
    GPK = G // K
    from contextlib import ExitStack
    with tile.TileContext(nc) as tc, ExitStack() as ctx:
        singles = ctx.enter_context(tc.tile_pool(name="singles", bufs=1))
        feat_pool = ctx.enter_context(tc.tile_pool(name="feats", bufs=3))
        y_pool = ctx.enter_context(tc.tile_pool(name="y", bufs=4))
        sc_pool = ctx.enter_context(tc.tile_pool(name="sc", bufs=4))
        small = ctx.enter_context(tc.tile_pool(name="small", bufs=4))
        mlp_pool = ctx.enter_context(tc.tile_pool(name="mlp", bufs=4))
        ps_y = ctx.enter_context(tc.tile_pool(name="ps_y", bufs=3, space="PSUM"))
        ps_u = ctx.enter_context(tc.tile_pool(name="ps_u", bufs=1, space="PSUM"))
        ps_xt = ctx.enter_context(tc.tile_pool(name="ps_xt", bufs=2, space="PSUM"))
        ps_mlp = ctx.enter_context(tc.tile_pool(name="ps_mlp", bufs=2, space="PSUM"))

        def load(name, shape, dt):
            t = singles.tile(shape, dt, tag=name)
            nc.sync.dma_start(out=t[:], in_=io[name])
            return t

        comb = load("comb", [128, 16], BF16)
        ident = load("ident", [128, 128], BF16)
        wfb = load("wfb", [64, 64], BF16)
        w1 = load("w1", [128, 64], BF16)
        w2 = load("w2", [64, 32], BF16)
        w3 = load("w3", [32, 1], BF16)
        b1 = load("b1", [64, 1], F32)
        b2 = load("b2", [32, 1], F32)
        b3 = load("b3", [1, 1], F32)
        idx = load("idx", [128, G * T], I32)
        rat = load("rat", [128, G * T], F32)
        tidx = load("tidx", [128, K], I32)

        # pooling weights: w = rating - 3, denom = sum_h |w| per batch row
        wv = singles.tile([128, G * T], F32)
        nc.vector.tensor_scalar_add(out=wv[:], in0=rat[:], scalar1=-3.0)
        wv_bf = singles.tile([128, G * T], BF16)
        nc.vector.tensor_copy(out=wv_bf[:], in_=wv[:])
        aw = singles.tile([128, G * T], F32)
        nc.scalar.activation(out=aw[:], in_=wv[:], func=AF.Abs)
        awr = singles.tile([128, G], F32)
        nc.vector.tensor_reduce(
            out=awr[:], in_=aw[:].rearrange("p (g t) -> p g t", t=T),
            axis=mybir.AxisListType.X, op=ALU.add,
        )
        awr_bf = singles.tile([128, G], BF16)
        nc.vector.tensor_copy(out=awr_bf[:], in_=awr[:])
        denom_ps = ps_mlp.tile([16, G], F32, tag="mlp")
        nc.tensor.matmul(out=denom_ps[:], lhsT=comb[:], rhs=awr_bf[:],
                         start=True, stop=True)
        denom = singles.tile([16, G], F32)
        nc.vector.tensor_scalar_add(out=denom[:], in0=denom_ps[:], scalar1=1e-8)
        invd = singles.tile([16, G], F32)
        nc.vector.reciprocal(out=invd[:], in_=denom[:])

        out_sb = singles.tile([1, Bc], F32)

        for k in range(K):
            xt_ps = ps_xt.tile([128, 128], BF16, tag="xt")

            # target rep for this MLP tile of 128 batch rows
            tfT = mlp_pool.tile([64, 128], BF16, tag="tfT")
            nc.sync.dma_start(out=tfT[:], in_=io["tfeatsT"][k, :, :])
            t_ps = ps_y.tile([128, 64], F32, tag="y")
            nc.tensor.matmul(out=t_ps[:], lhsT=tfT[:], rhs=wfb[:],
                             start=True, stop=True)
            trep = y_pool.tile([128, 64], BF16, tag="trep")
            nc.scalar.activation(out=trep[:], in_=t_ps[:], func=AF.Copy)
            if do_gather:
                gi = nc.gpsimd.indirect_dma_start(
                    out=trep[:], out_offset=None, in_=io["table"],
                    in_offset=IndirectOffsetOnAxis(ap=tidx[:, k:k + 1], axis=0),
                    compute_op=ALU.add,
                )
                gi.ins.queue = _rr_queue()
            trep2 = sc_pool.tile([128, 64], BF16, tag="trep2")
            nc.vector.tensor_scalar_max(out=trep2[:], in0=trep[:], scalar1=0.0)
            nc.tensor.transpose(out=xt_ps[64:128, :], in_=trep2[:],
                                identity=ident[:])

            u_ps = ps_u.tile([16, GPK * 64], F32, tag="u")
            for gl in range(GPK):
                g = k * GPK + gl
                if g % BG == 0:
                    fT = feat_pool.tile([64, BG * T * 128], BF16)
                    nc.sync.dma_start(
                        out=fT[:].rearrange("f (b c) -> f b c", b=BG),
                        in_=io["featsT"][g:g + BG, :, :].rearrange(
                            "b f c -> f b c"))
                fb = (g % BG) * T * 128
                yps = ps_y.tile([128, T * 64], F32, tag="y")
                for t in range(T):
                    nc.tensor.matmul(
                        out=yps[:, t * 64:(t + 1) * 64],
                        lhsT=fT[:, fb + t * 128:fb + (t + 1) * 128],
                        rhs=wfb[:], start=True, stop=True,
                    )
                yslab = y_pool.tile([128, T * 64], BF16)
                nc.scalar.activation(out=yslab[:], in_=yps[:], func=AF.Copy)
                if do_gather:
                    for t in range(T):
                        gi = nc.gpsimd.indirect_dma_start(
                            out=yslab[:, t * 64:(t + 1) * 64],
                            out_offset=None, in_=io["table"],
                            in_offset=IndirectOffsetOnAxis(
                                ap=idx[:, g * T + t:g * T + t + 1], axis=0),
                            compute_op=ALU.add,
                        )
                        gi.ins.queue = _rr_queue()
                scaled = sc_pool.tile([128, T, 64], BF16)
                nc.vector.scalar_tensor_tensor(
                    out=scaled[:, :, :],
                    in0=yslab[:].rearrange("p (t e) -> p t e", e=64),
                    scalar=0.0,
                    in1=wv_bf[:, g * T:(g + 1) * T].to_broadcast(
                        [128, T, 64]),
                    op0=ALU.max, op1=ALU.mult,
                )
                for t in range(T):
                    nc.tensor.matmul(
                        out=u_ps[:, gl * 64:(gl + 1) * 64], lhsT=comb[:],
                        rhs=scaled[:, t, :],
                        start=(t == 0), stop=(t == T - 1),
                    )
            uslab = small.tile([16, GPK, 64], BF16, tag="uslab")
            nc.vector.scalar_tensor_tensor(
                out=uslab[:, :, :],
                in0=u_ps[:].rearrange("p (g e) -> p g e", e=64),
                scalar=0.0,
                in1=invd[:, k * GPK:(k + 1) * GPK].to_broadcast(
                    [16, GPK, 64]),
                op0=ALU.bypass, op1=ALU.mult,
            )
            for gl in range(GPK):
                nc.tensor.transpose(
                    out=xt_ps[0:64, gl * 16:(gl + 1) * 16],
                    in_=uslab[:, gl, :],
                    identity=ident[:16, :16],
                )

            xt_sb = mlp_pool.tile([128, 128], BF16)
            nc.vector.tensor_copy(out=xt_sb[:], in_=xt_ps[:])
            p1 = ps_mlp.tile([64, 128], F32, tag="mlp")
            nc.tensor.matmul(out=p1[:], lhsT=w1[:], rhs=xt_sb[:],
                             start=True, stop=True)
            h1 = mlp_pool.tile([64, 128], BF16)
            nc.scalar.activation(out=h1[:], in_=p1[:], func=AF.Relu,
                                 bias=b1[:], scale=1.0)
            p2 = ps_mlp.tile([32, 128], F32, tag="mlp")
            nc.tensor.matmul(out=p2[:], lhsT=w2[:], rhs=h1[:],
                             start=True, stop=True)
            h2 = mlp_pool.tile([32, 128], BF16)
            nc.scalar.activation(out=h2[:], in_=p2[:], func=AF.Relu,
                                 bias=b2[:], scale=1.0)
            p3 = ps_mlp.tile([1, 128], F32, tag="mlp")
            nc.tensor.matmul(out=p3[:], lhsT=w3[:], rhs=h2[:],
                             start=True, stop=True)
            nc.scalar.activation(
                out=out_sb[:, k * 128:(k + 1) * 128], in_=p3[:],
                func=AF.Identity, bias=b3[:], scale=1.0,
            )

        nc.sync.dma_start(out=io["out"], in_=out_sb[:])


_NC_CACHE = {}


def _get_nc(do_gather=True):
    if do_gather in _NC_CACHE:
        return _NC_CACHE[do_gather]
    nc = bass.Bass(num_swdge_queues=4)
    io = {}
    def din(name, shape, dt):
        io[name] = nc.dram_tensor(name, shape, dt, kind="ExternalInput").ap()
    din("table", [V, 64], BF16)
    din("featsT", [G, 64, T * 128], BF16)
    din("idx", [128, G * T], I32)
    din("rat", [128, G * T], F32)
    din("tfeatsT", [K, 64, 128], BF16)
    din("tidx", [128, K], I32)
    din("comb", [128, 16], BF16)
    din("ident", [128, 128], BF16)
    din("wfb", [64, 64], BF16)
    din("w1", [128, 64], BF16)
    din("w2", [64, 32], BF16)
    din("w3", [32, 1], BF16)
    din("b1", [64, 1], F32)
    din("b2", [32, 1], F32)
    din("b3", [1, 1], F32)
    io["out"] = nc.dram_tensor("out", [Bc], F32, kind="ExternalOutput").ap()
    build_kernel(nc, io, do_gather=do_gather)
    _split_excess_waits(nc)
    _NC_CACHE[do_gather] = nc
    return nc


# ---------------------------------------------------------------------------
# Host-side shard prep


def _prep_shared(embed_table, fusion_w, fusion_b, w1, b1, w2, b2, w3, b3):
    table2 = embed_table.astype(np.float32) @ fusion_w[:64].astype(np.float32) \
        + fusion_b.astype(np.float32)
    comb = np.zeros((128, 16), bf16)
    comb[np.arange(128), np.arange(128) % 16] = 1.0
    return {
        "table": np.ascontiguousarray(table2.astype(bf16)),
        "comb": comb,
        "ident": np.eye(128, dtype=bf16),
        "wfb": np.ascontiguousarray(fusion_w[64:].astype(bf16)),
        "w1": np.ascontiguousarray(w1.astype(bf16)),
        "w2": np.ascontiguousarray(w2.astype(bf16)),
        "w3": np.ascontiguousarray(w3.astype(bf16)),
        "b1": np.ascontiguousarray(b1.reshape(64, 1).astype(np.float32)),
        "b2": np.ascontiguousarray(b2.reshape(32, 1).astype(np.float32)),
        "b3": np.ascontiguousarray(b3.reshape(1, 1).astype(np.float32)),
    }


def _prep_core(hist_indices, hist_features, hist_ratings, target_indices,
               target_features):
    Bc = hist_indices.shape[0]
    G = Bc // 16
    K = Bc // 128
    HP = T * 8
    idx_p = np.zeros((Bc, HP), np.int32)
    idx_p[:, :H] = hist_indices
    rat_p = np.full((Bc, HP), 3.0, np.float32)
    rat_p[:, :H] = hist_ratings
    feat_p = np.zeros((Bc, HP, 64), np.float32)
    feat_p[:, :H, :] = hist_features

    # [g, m, t, j, ...] with b = 16g + m, h = 8t + j; partition p = 16j + m
    vi = idx_p.reshape(G, 16, T, 8)
    idx_dev = np.ascontiguousarray(
        vi.transpose(3, 1, 0, 2).reshape(128, G * T), np.int32)
    vr = rat_p.reshape(G, 16, T, 8)
    rat_dev = np.ascontiguousarray(
        vr.transpose(3, 1, 0, 2).reshape(128, G * T), np.float32)
    vf = feat_p.reshape(G, 16, T, 8, 64).astype(bf16)
    featsT = np.ascontiguousarray(
        vf.transpose(0, 4, 2, 3, 1).reshape(G, 64, T * 128))
    tidx = np.ascontiguousarray(
        target_indices.astype(np.int64).reshape(K, 128).T, np.int32)
    tfeatsT = np.ascontiguousarray(
        target_features.reshape(K, 128, 64).transpose(0, 2, 1).astype(bf16))
    return {
        "featsT": featsT,
        "idx": idx_dev,
        "rat": rat_dev,
        "tfeatsT": tfeatsT,
        "tidx": tidx,
    }


def prep_in_maps(inputs):
    shared = _prep_shared(
        np.asarray(inputs["embed_table"], np.float32),
        np.asarray(inputs["fusion_w"], np.float32),
        np.asarray(inputs["fusion_b"], np.float32),
        np.asarray(inputs["w1"], np.float32),
        np.asarray(inputs["b1"], np.float32),
        np.asarray(inputs["w2"], np.float32),
        np.asarray(inputs["b2"], np.float32),
        np.asarray(inputs["w3"], np.float32),
        np.asarray(inputs["b3"], np.float32),
    )
    hi = np.asarray(inputs["hist_indices"])
    hf = np.asarray(inputs["hist_features"], np.float32)
    hr = np.asarray(inputs["hist_ratings"], np.float32)
    ti = np.asarray(inputs["target_indices"])
    tf = np.asarray(inputs["target_features"], np.float32)
    in_maps = []
    for c in range(N_CORES):
        s = slice(c * Bc, (c + 1) * Bc)
        m = dict(shared)
        m.update(_prep_core(hi[s], hf[s], hr[s], ti[s], tf[s]))
        in_maps.append(m)
    return in_maps


_RUNNER = None


def _get_runner():
    """Persistent jitted 8-core runner (mirrors bass2jax.run_bass_via_pjrt but
    cached, so repeat kernel() calls skip retracing/recompiling)."""
    global _RUNNER
    if _RUNNER is not None:
        return _RUNNER
    import jax
    from jax.sharding import Mesh, PartitionSpec
    from jax.experimental.shard_map import shard_map
    from concourse.bass2jax import (
        _bass_exec_p, install_neuronx_cc_hook, partition_id_tensor)

    nc = _get_nc()
    install_neuronx_cc_hook()
    partition_name = nc.partition_id_tensor.name if nc.partition_id_tensor else None
    in_names, out_names, out_avals, zero_outs = [], [], [], []
    for alloc in nc.m.functions[0].allocations:
        if not isinstance(alloc, mybir.MemoryLocationSet):
            continue
        name = alloc.memorylocations[0].name
        if alloc.kind == "ExternalInput":
            if name != partition_name:
                in_names.append(name)
        elif alloc.kind == "ExternalOutput":
            out_names.append(name)
            shape = tuple(alloc.tensor_shape)
            dtype = mybir.dt.np(alloc.dtype)
            out_avals.append(jax.core.ShapedArray(shape, dtype))
            zero_outs.append(np.zeros(shape, dtype))
    n_params = len(in_names)
    all_names = list(in_names) + list(out_names)
    if partition_name is not None:
        all_names.append(partition_name)
    donate = tuple(range(n_params, n_params + len(out_names)))

    def _body(*args):
        operands = list(args)
        if partition_name is not None:
            operands.append(partition_id_tensor())
        return tuple(_bass_exec_p.bind(
            *operands,
            out_avals=tuple(out_avals),
            in_names=tuple(all_names),
            out_names=tuple(out_names),
            lowering_input_output_aliases=(),
            sim_require_finite=True,
            sim_require_nnan=True,
            nc=nc,
        ))

    devices = jax.devices()[:N_CORES]
    mesh = Mesh(np.asarray(devices), ("core",))
    sharded = jax.jit(
        shard_map(_body, mesh=mesh,
                  in_specs=(PartitionSpec("core"),) * (n_params + len(out_names)),
                  out_specs=(PartitionSpec("core"),) * len(out_names),
                  check_rep=False),
        donate_argnums=donate, keep_unused=True,
    )

    def run(in_maps):
        per_core = [[np.asarray(m[n]) for n in in_names] for m in in_maps]
        concat_in = [
            np.concatenate([per_core[c][i] for c in range(N_CORES)], axis=0)
            for i in range(n_params)
        ]
        concat_zeros = [
            np.zeros((N_CORES * z.shape[0], *z.shape[1:]), z.dtype)
            for z in zero_outs
        ]
        outs = sharded(*concat_in, *concat_zeros)
        return np.asarray(outs[out_names.index("out")]).reshape(-1)

    _RUNNER = run
    return run


def kernel(**inputs) -> np.ndarray:
    run = _get_runner()
    in_maps = prep_in_maps(inputs)
    return run(in_maps).astype(np.float32)

